# revision 30
# baseline (speedup 1.0000x reference)
"""Trainium2 Bass kernel: single-head causal attention (nn_Head).

Reference computation (per batch b):
    q = x @ Wq.T; k = x @ Wk.T; v = x @ Wv.T          # [T, H]
    S = q @ k.T * D**-0.5, causal-masked               # [T, T]
    P = softmax(S, axis=-1)
    out = P @ v                                        # [T, H]

Shapes: B=16, T=1024, D=768, H=64. f32 in / f32 out.

Sharding: pure data-parallel over batch. 8 cores x 2 batches each; weights
replicated; no collectives. Host shards x, gathers out.

Layout strategy: the host marshals inputs into the layouts the PE array
wants, so the device does almost no transposing:
  - x is pre-transposed on the host to x^T [B, D, T] and cast to bf16;
    each core DMAs its [BL, D, T] slice straight into SBUF with d on
    partitions, in k-slice chunks so matmuls start before the full load.
  - Wq/Wk/Wv are pre-transposed and fused into one [D, 192] bf16 matrix.
    One stationary [d, (qh|kh)] operand yields q^T and k^T from a single
    accumulation pass; v is computed as v^T (large-N matmuls, small
    LDWEIGHTS) then turned natural by 8 PE transposes.
  - S^T [s, t] blocks (s on partitions) feed softmax without transposes:
    exp via ScalarE writes P^T directly; out^T = [v|1]^T @ P^T puts the
    row-sums in an extra out^T row (appended ones-column of v). The final
    divide + transpose happen on the host at gather time.
  - The PE executes its queue in order, so the two batches are
    hand-interleaved: batch 1's projection matmuls are issued between
    batch 0's S-block groups (filling exp-wait stalls), batch 1's S
    between batch 0's out^T accumulation, etc. A short burst of dummy
    matmuls warms the PE clock gate while the first DMA is in flight.
"""

import os
import sys

for _p in ("/opt/trn_rl_repo", "/root/.axon_site/_ro/trn_rl_repo"):
    if os.path.isdir(_p) and _p not in sys.path:
        sys.path.insert(0, _p)

import numpy as np

import concourse.bass as bass
import concourse.bacc as bacc
import concourse.mybir as mybir
import concourse.tile as tile
from contextlib import ExitStack
from concourse.masks import make_identity, make_lower_triangular

B, T, D, H = 16, 1024, 768, 64
NCORES = 8
BL = B // NCORES          # batches per core
TT = T // 128             # 8 t-tiles
KD = D // 128             # 6 d-slices
F32 = mybir.dt.float32
BF16 = mybir.dt.bfloat16
SCALE = float(D) ** -0.5
NEG = -1e30
NP_BF16 = mybir.dt.np(BF16)

# S^T column chunks per s-block j (each fits one PSUM bank):
# j < 4 -> [(128j, 512), (512, 1024)]; j >= 4 -> [(128j, 1024)]
def _s_chunks(j):
    lo = 128 * j
    return [(lo, 512), (512, T)] if lo < 512 else [(lo, T)]


def build_nc():
    nc = bacc.Bacc()
    xT = nc.declare_dram_parameter("xT", [BL, D, T], BF16, isOutput=False)[:]
    w = nc.declare_dram_parameter("w", [D, 3 * H], BF16, isOutput=False)[:]
    out = nc.declare_dram_parameter("outT", [BL, H + 1, T], F32, isOutput=True)[:]

    with tile.TileContext(nc) as tc, ExitStack() as ctx:
        const = ctx.enter_context(tc.tile_pool(name="const", bufs=1))
        wpool = ctx.enter_context(tc.tile_pool(name="wpool", bufs=1))
        xpool = ctx.enter_context(tc.tile_pool(name="xpool", bufs=2))
        mid = ctx.enter_context(tc.tile_pool(name="mid", bufs=2))
        ptp = ctx.enter_context(tc.tile_pool(name="ptp", bufs=2))
        outp = ctx.enter_context(tc.tile_pool(name="outp", bufs=2))
        ps_qk = ctx.enter_context(tc.tile_pool(name="ps_qk", bufs=1, space="PSUM"))
        ps_v = ctx.enter_context(tc.tile_pool(name="ps_v", bufs=1, space="PSUM"))
        ps_s = ctx.enter_context(tc.tile_pool(name="ps_s", bufs=2, space="PSUM"))
        ps_o = ctx.enter_context(tc.tile_pool(name="ps_o", bufs=1, space="PSUM"))

        # additive causal mask for the diagonal [s,t] block of S^T
        mask = const.tile([128, 128], F32)
        make_lower_triangular(nc, mask, val=NEG, diag=False)

        # ---- PE warm-up while the first x^T DMA is in flight ----
        wz = const.tile([128, 512], BF16)
        nc.gpsimd.memset(wz, 0.0)
        wps = ps_o.tile([128, 512], F32, name="wps", tag="ps_o")
        for _ in range(4):
            nc.tensor.matmul(wps, wz[:, 0:128], wz, start=True, stop=True)

        # ---- input DMAs: weights + x^T second T-halves on the scalar
        # HWDGE ring, x^T first T-halves on the sync ring (parallel rings;
        # the c0 matmul groups only need the first half of each x^T) ----
        xts = []
        for b in range(BL):
            xts.append(xpool.tile([128, KD, T], BF16, name=f"xt{b}", tag=f"xt{b}"))
        # priority: only w + the first T-half of x^T[0] in flight initially,
        # split so the first matmul group can start as early as possible.
        w_s = wpool.tile([128, KD, 3 * H], BF16)
        nc.scalar.dma_start(out=w_s, in_=w.rearrange("(k p) h -> p k h", p=128))
        xv0 = xT[0].rearrange("(k p) t -> p k t", p=128)
        nc.sync.dma_start(out=xts[0][:, 0:3, 0:512], in_=xv0[:, 0:3, 0:512])
        nc.sync.dma_start(out=xts[0][:, 3:6, 0:512], in_=xv0[:, 3:6, 0:512])
        nc.scalar.dma_start(out=xts[0][:, :, 512:T], in_=xv0[:, :, 512:T])
        xv1 = xT[1].rearrange("(k p) t -> p k t", p=128)
        nc.sync.dma_start(out=xts[1][:, :, 0:512], in_=xv1[:, :, 0:512])
        nc.scalar.dma_start(out=xts[1][:, :, 512:T], in_=xv1[:, :, 512:T])

        # per-batch tiles
        qT, kT, vs, pt, po, ot = {}, {}, {}, {}, {}, {}
        for b in range(BL):
            qT[b] = mid.tile([H, T], BF16, name=f"qT{b}", tag="qT")
            kT[b] = mid.tile([H, T], BF16, name=f"kT{b}", tag="kT")
            vs[b] = mid.tile([128, TT, H + 1], BF16, name=f"vs{b}", tag="vs")
            pt[b] = ptp.tile([128, TT, T], BF16, name=f"pt{b}", tag="pt")
            ot[b] = outp.tile([H + 1, T], F32, name=f"ot{b}", tag="ot")

        def qk_group(b, c):
            pqk = ps_qk.tile([128, 512], F32, name="pqk", tag="ps_qk")
            for k in range(KD):
                nc.tensor.matmul(
                    pqk, w_s[:, k, 0:128], xts[b][:, k, 512 * c:512 * (c + 1)],
                    start=(k == 0), stop=(k == KD - 1),
                )
            nc.vector.tensor_copy(qT[b][:, 512 * c:512 * (c + 1)], pqk[0:H, :])
            nc.vector.tensor_copy(kT[b][:, 512 * c:512 * (c + 1)], pqk[H:128, :])

        v_psum = {}

        def v_group(b, ilist):
            # v natural [t, h] directly: stationary x^T tile, N=64
            if b not in v_psum:
                v_psum[b] = ps_v.tile([128, TT, H], F32, name=f"pv{b}", tag="ps_v")
            pv = v_psum[b]
            for i in ilist:
                for k in range(KD):
                    nc.tensor.matmul(
                        pv[:, i, :], xts[b][:, k, 128 * i:128 * (i + 1)],
                        w_s[:, k, 128:192],
                        start=(k == 0), stop=(k == KD - 1),
                    )

        def vs_copy(b):
            nc.vector.tensor_copy(vs[b][:, :, 0:H], v_psum[b])
            nc.gpsimd.memset(vs[b][:, :, H:H + 1], 1.0)

        def s_block(b, j):
            # j < 4: one [128, 1024] tile holding columns [128j, 1024) at
            # absolute offsets; single merged exp.
            lo = 128 * j
            ps = ps_s.tile([128, T], F32, name="ps", tag="ps_s")
            nc.tensor.matmul(
                ps[:, lo:512], kT[b][:, lo:lo + 128], qT[b][:, lo:512],
                start=True, stop=True,
            )
            nc.tensor.matmul(
                ps[:, 512:T], kT[b][:, lo:lo + 128], qT[b][:, 512:T],
                start=True, stop=True,
            )
            nc.vector.tensor_add(ps[:, lo:lo + 128], ps[:, lo:lo + 128], mask)
            nc.scalar.activation(
                pt[b][:, j, lo:T], ps[:, lo:T],
                mybir.ActivationFunctionType.Exp, scale=SCALE,
            )

        def s_pair(b, jp):
            # j >= 4: blocks (jp, jp+1) share one tile over columns
            # [512, 1024) each; below-diagonal columns of jp+1 are junk
            # that AV never reads. One paired exp.
            ps = ps_s.tile([128, T], F32, name="ps", tag="ps_s")
            for h, j in enumerate((jp, jp + 1)):
                lo = 128 * j
                nc.tensor.matmul(
                    ps[:, 512 * h:512 * (h + 1)], kT[b][:, lo:lo + 128],
                    qT[b][:, 512:T],
                    start=True, stop=True,
                )
                nc.vector.tensor_add(
                    ps[:, 512 * h + lo - 512:512 * h + lo - 384],
                    ps[:, 512 * h + lo - 512:512 * h + lo - 384],
                    mask,
                )
            nc.scalar.activation(
                pt[b][:, jp:jp + 2, 512:T],
                ps.rearrange("p (j t) -> p j t", j=2),
                mybir.ActivationFunctionType.Exp, scale=SCALE,
            )

        def av_chunk(b, j, s0, s1):
            # out^T column region accumulates over j; bank A [128j, 512)
            # sees j=0..3, bank B [512, 1024) sees j=0..7.
            if s1 == 512:
                first, last = (j == 0), (j == 3)
            else:
                first, last = (j == 0), (j == 7)
            nc.tensor.matmul(
                po[b][:, s0:s1], vs[b][:, j, :], pt[b][:, j, s0:s1],
                start=first, stop=last, skip_group_check=True,
            )

        def finish(b):
            nc.vector.tensor_copy(ot[b], po[b])
            eng = nc.sync if b == 0 else nc.scalar
            eng.dma_start(out=out[b], in_=ot[b])

        SC = [(j, s0, s1) for j in range(TT) for (s0, s1) in _s_chunks(j)]

        def s_step(b, n):
            # n-th of 6 S steps: blocks j=0..3 then pairs (4,5), (6,7)
            if n < 4:
                s_block(b, n)
            else:
                s_pair(b, 4 if n == 4 else 6)

        def av_j(b, j):
            for (s0, s1) in _s_chunks(j):
                av_chunk(b, j, s0, s1)

        # ---- hand-interleaved issue order (PE executes in order) ----
        qk_group(0, 0)
        v_group(0, [0, 1, 2, 3])
        qk_group(0, 1)
        v_group(0, [4, 5, 6, 7])
        vs_copy(0)
        s_step(0, 0)
        s_step(0, 1)
        s_step(0, 2)
        qk_group(1, 0)
        s_step(0, 3)
        s_step(0, 4)
        qk_group(1, 1)
        s_step(0, 5)
        po[0] = ps_o.tile([H + 1, T], F32, name="po0", tag="ps_o")
        for n in range(6):
            s_step(1, n)
            if n < 4:
                av_j(0, n)
                v_group(1, [n])
            elif n == 4:
                av_j(0, 4)
                av_j(0, 5)
                v_group(1, [4, 5])
            else:
                av_j(0, 6)
                av_j(0, 7)
                v_group(1, [6, 7])
        vs_copy(1)
        finish(0)
        po[1] = ps_o.tile([H + 1, T], F32, name="po1", tag="ps_o")
        for j in range(TT):
            av_j(1, j)
        finish(1)

    nc.finalize()
    return nc


_NC_CACHE = {}


def _get_nc():
    if "nc" not in _NC_CACHE:
        _NC_CACHE["nc"] = build_nc()
    return _NC_CACHE["nc"]


def _make_in_maps(inputs):
    x = np.asarray(inputs["x"], dtype=np.float32)
    wq = np.asarray(inputs["Wq"], dtype=np.float32)
    wk = np.asarray(inputs["Wk"], dtype=np.float32)
    wv = np.asarray(inputs["Wv"], dtype=np.float32)
    # host-side input marshaling: transpose + cast into device layouts
    xT = np.ascontiguousarray(x.transpose(0, 2, 1)).astype(NP_BF16)    # [B, D, T]
    w = np.ascontiguousarray(
        np.concatenate([wq.T, wk.T, wv.T], axis=1)
    ).astype(NP_BF16)                                                  # [D, 3H]
    in_maps = []
    for c in range(NCORES):
        in_maps.append(
            {
                "xT": np.ascontiguousarray(xT[c * BL:(c + 1) * BL]),
                "w": w,
            }
        )
    return in_maps


def _assemble(results):
    # gather: device returns out^T with the softmax denominator as row H;
    # normalize and transpose back to [B, T, H]
    oT = np.concatenate([np.asarray(r["outT"], np.float32) for r in results], axis=0)
    o = oT[:, :H, :] / oT[:, H:H + 1, :]
    return np.ascontiguousarray(o.transpose(0, 2, 1)).astype(np.float32)


def kernel(**inputs):
    from concourse.bass_utils import run_bass_kernel_spmd

    nc = _get_nc()
    res = run_bass_kernel_spmd(nc, _make_in_maps(inputs), list(range(NCORES)))
    return _assemble(res.results)


if __name__ == "__main__":
    nc = build_nc()
    print("built OK")


# revision 32
# speedup vs baseline: 1.1373x; 1.1373x over previous
"""Trainium2 Bass kernel: single-head causal attention (nn_Head).

Reference computation (per batch b):
    q = x @ Wq.T; k = x @ Wk.T; v = x @ Wv.T          # [T, H]
    S = q @ k.T * D**-0.5, causal-masked               # [T, T]
    P = softmax(S, axis=-1)
    out = P @ v                                        # [T, H]

Shapes: B=16, T=1024, D=768, H=64. f32 in / f32 out.

Sharding: pure data-parallel over batch. 8 cores x 2 batches each; weights
replicated; no collectives. Host shards x, gathers out.

Layout strategy: the host marshals inputs into the layouts the PE array
wants, so the device does almost no transposing:
  - x is pre-transposed on the host to x^T [B, D, T] and cast to bf16;
    each core DMAs its [BL, D, T] slice straight into SBUF with d on
    partitions, in k-slice chunks so matmuls start before the full load.
  - Wq/Wk/Wv are pre-transposed and fused into one [D, 192] bf16 matrix.
    One stationary [d, (qh|kh)] operand yields q^T and k^T from a single
    accumulation pass; v is computed as v^T (large-N matmuls, small
    LDWEIGHTS) then turned natural by 8 PE transposes.
  - S^T [s, t] blocks (s on partitions) feed softmax without transposes:
    exp via ScalarE writes P^T directly; out^T = [v|1]^T @ P^T puts the
    row-sums in an extra out^T row (appended ones-column of v). The final
    divide + transpose happen on the host at gather time.
  - The PE executes its queue in order, so the two batches are
    hand-interleaved: batch 1's projection matmuls are issued between
    batch 0's S-block groups (filling exp-wait stalls), batch 1's S
    between batch 0's out^T accumulation, etc. A short burst of dummy
    matmuls warms the PE clock gate while the first DMA is in flight.
"""

import os
import sys

for _p in ("/opt/trn_rl_repo", "/root/.axon_site/_ro/trn_rl_repo"):
    if os.path.isdir(_p) and _p not in sys.path:
        sys.path.insert(0, _p)

import numpy as np

import concourse.bass as bass
import concourse.bacc as bacc
import concourse.mybir as mybir
import concourse.tile as tile
from contextlib import ExitStack
from concourse.masks import make_identity, make_lower_triangular

B, T, D, H = 16, 1024, 768, 64
NCORES = 8
BL = B // NCORES          # batches per core
TT = T // 128             # 8 t-tiles
KD = D // 128             # 6 d-slices
F32 = mybir.dt.float32
BF16 = mybir.dt.bfloat16
SCALE = float(D) ** -0.5
NEG = -1e30
NP_BF16 = mybir.dt.np(BF16)

# S^T column chunks per s-block j (each fits one PSUM bank):
# j < 4 -> [(128j, 512), (512, 1024)]; j >= 4 -> [(128j, 1024)]
def _s_chunks(j):
    lo = 128 * j
    return [(lo, 512), (512, T)] if lo < 512 else [(lo, T)]


def build_nc():
    nc = bacc.Bacc()
    xT = nc.declare_dram_parameter("xT", [BL, D, T], BF16, isOutput=False)[:]
    w = nc.declare_dram_parameter("w", [D, 3 * H], BF16, isOutput=False)[:]
    out = nc.declare_dram_parameter("outT", [BL, H + 1, T], F32, isOutput=True)[:]

    with tile.TileContext(nc) as tc, ExitStack() as ctx:
        const = ctx.enter_context(tc.tile_pool(name="const", bufs=1))
        wpool = ctx.enter_context(tc.tile_pool(name="wpool", bufs=1))
        xpool = ctx.enter_context(tc.tile_pool(name="xpool", bufs=2))
        mid = ctx.enter_context(tc.tile_pool(name="mid", bufs=2))
        ptp = ctx.enter_context(tc.tile_pool(name="ptp", bufs=2))
        outp = ctx.enter_context(tc.tile_pool(name="outp", bufs=2))
        ps_qk = ctx.enter_context(tc.tile_pool(name="ps_qk", bufs=1, space="PSUM"))
        ps_v = ctx.enter_context(tc.tile_pool(name="ps_v", bufs=1, space="PSUM"))
        ps_s = ctx.enter_context(tc.tile_pool(name="ps_s", bufs=2, space="PSUM"))
        ps_o = ctx.enter_context(tc.tile_pool(name="ps_o", bufs=1, space="PSUM"))

        # additive causal mask for the diagonal [s,t] block of S^T
        mask = const.tile([128, 128], F32)
        make_lower_triangular(nc, mask, val=NEG, diag=False)

        # ---- PE warm-up while the first x^T DMA is in flight ----
        wz = const.tile([128, 512], BF16)
        nc.gpsimd.memset(wz, 0.0)
        wps = ps_o.tile([128, 512], F32, name="wps", tag="ps_o")
        for _ in range(4):
            nc.tensor.matmul(wps, wz[:, 0:128], wz, start=True, stop=True)

        # ---- input DMAs: weights + x^T second T-halves on the scalar
        # HWDGE ring, x^T first T-halves on the sync ring (parallel rings;
        # the c0 matmul groups only need the first half of each x^T) ----
        xts = []
        for b in range(BL):
            xts.append(xpool.tile([128, KD, T], BF16, name=f"xt{b}", tag=f"xt{b}"))
        # priority: batch 0's x^T (both halves) + w complete first — they
        # gate the first S/exp chain. Each ring is FIFO, the two rings share
        # bandwidth, so batch 1's loads queue behind batch 0's on each ring.
        w_s = wpool.tile([128, KD, 3 * H], BF16)
        nc.scalar.dma_start(out=w_s, in_=w.rearrange("(k p) h -> p k h", p=128))
        xv0 = xT[0].rearrange("(k p) t -> p k t", p=128)
        nc.sync.dma_start(out=xts[0][:, :, 0:512], in_=xv0[:, :, 0:512])
        nc.scalar.dma_start(out=xts[0][:, :, 512:T], in_=xv0[:, :, 512:T])
        xv1 = xT[1].rearrange("(k p) t -> p k t", p=128)
        nc.sync.dma_start(out=xts[1][:, :, 0:512], in_=xv1[:, :, 0:512])
        nc.scalar.dma_start(out=xts[1][:, :, 512:T], in_=xv1[:, :, 512:T])

        # per-batch tiles
        qT, kT, vs, pt, po, ot = {}, {}, {}, {}, {}, {}
        for b in range(BL):
            qT[b] = mid.tile([H, T], BF16, name=f"qT{b}", tag="qT")
            kT[b] = mid.tile([H, T], BF16, name=f"kT{b}", tag="kT")
            vs[b] = mid.tile([128, TT, H + 1], BF16, name=f"vs{b}", tag="vs")
            pt[b] = ptp.tile([128, TT, T], BF16, name=f"pt{b}", tag="pt")
            ot[b] = outp.tile([H + 1, T], F32, name=f"ot{b}", tag="ot")

        def qk_group(b, c):
            pqk = ps_qk.tile([128, 512], F32, name="pqk", tag="ps_qk")
            for k in range(KD):
                nc.tensor.matmul(
                    pqk, w_s[:, k, 0:128], xts[b][:, k, 512 * c:512 * (c + 1)],
                    start=(k == 0), stop=(k == KD - 1),
                )
            nc.vector.tensor_copy(qT[b][:, 512 * c:512 * (c + 1)], pqk[0:H, :])
            nc.vector.tensor_copy(kT[b][:, 512 * c:512 * (c + 1)], pqk[H:128, :])

        v_psum = {}

        def v_group(b, ilist):
            # v natural [t, h] directly: stationary x^T tile, N=64
            if b not in v_psum:
                v_psum[b] = ps_v.tile([128, TT, H], F32, name=f"pv{b}", tag="ps_v")
            pv = v_psum[b]
            for i in ilist:
                for k in range(KD):
                    nc.tensor.matmul(
                        pv[:, i, :], xts[b][:, k, 128 * i:128 * (i + 1)],
                        w_s[:, k, 128:192],
                        start=(k == 0), stop=(k == KD - 1),
                    )

        def vs_copy(b):
            nc.vector.tensor_copy(vs[b][:, :, 0:H], v_psum[b])
            nc.gpsimd.memset(vs[b][:, :, H:H + 1], 1.0)

        def s_block(b, j):
            # j < 4: one [128, 1024] tile holding columns [128j, 1024) at
            # absolute offsets; single merged exp.
            lo = 128 * j
            ps = ps_s.tile([128, T], F32, name="ps", tag="ps_s")
            nc.tensor.matmul(
                ps[:, lo:512], kT[b][:, lo:lo + 128], qT[b][:, lo:512],
                start=True, stop=True,
            )
            nc.tensor.matmul(
                ps[:, 512:T], kT[b][:, lo:lo + 128], qT[b][:, 512:T],
                start=True, stop=True,
            )
            nc.vector.tensor_add(ps[:, lo:lo + 128], ps[:, lo:lo + 128], mask)
            nc.scalar.activation(
                pt[b][:, j, lo:T], ps[:, lo:T],
                mybir.ActivationFunctionType.Exp, scale=SCALE,
            )

        def s_pair(b, jp):
            # j >= 4: blocks (jp, jp+1) share one tile over columns
            # [512, 1024) each; below-diagonal columns of jp+1 are junk
            # that AV never reads. One paired exp.
            ps = ps_s.tile([128, T], F32, name="ps", tag="ps_s")
            for h, j in enumerate((jp, jp + 1)):
                lo = 128 * j
                nc.tensor.matmul(
                    ps[:, 512 * h:512 * (h + 1)], kT[b][:, lo:lo + 128],
                    qT[b][:, 512:T],
                    start=True, stop=True,
                )
                nc.vector.tensor_add(
                    ps[:, 512 * h + lo - 512:512 * h + lo - 384],
                    ps[:, 512 * h + lo - 512:512 * h + lo - 384],
                    mask,
                )
            nc.scalar.activation(
                pt[b][:, jp:jp + 2, 512:T],
                ps.rearrange("p (j t) -> p j t", j=2),
                mybir.ActivationFunctionType.Exp, scale=SCALE,
            )

        def av_chunk(b, j, s0, s1):
            # out^T column region accumulates over j; bank A [128j, 512)
            # sees j=0..3, bank B [512, 1024) sees j=0..7.
            if s1 == 512:
                first, last = (j == 0), (j == 3)
            else:
                first, last = (j == 0), (j == 7)
            nc.tensor.matmul(
                po[b][:, s0:s1], vs[b][:, j, :], pt[b][:, j, s0:s1],
                start=first, stop=last, skip_group_check=True,
            )

        def finish(b):
            nc.vector.tensor_copy(ot[b], po[b])
            eng = nc.sync if b == 0 else nc.scalar
            eng.dma_start(out=out[b], in_=ot[b])

        SC = [(j, s0, s1) for j in range(TT) for (s0, s1) in _s_chunks(j)]

        def s_step(b, n):
            # n-th of 6 S steps: blocks j=0..3 then pairs (4,5), (6,7)
            if n < 4:
                s_block(b, n)
            else:
                s_pair(b, 4 if n == 4 else 6)

        def av_j(b, j):
            for (s0, s1) in _s_chunks(j):
                av_chunk(b, j, s0, s1)

        # ---- hand-interleaved issue order (PE executes in order).
        # The S0 -> exp chain has absolute priority; the v projections and
        # batch 1's work are pure fill behind it.
        qk_group(0, 0)
        qk_group(0, 1)
        s_step(0, 0)
        s_step(0, 1)
        v_group(0, [0, 1])
        s_step(0, 2)
        v_group(0, [2, 3])
        s_step(0, 3)
        v_group(0, [4, 5])
        s_step(0, 4)
        v_group(0, [6, 7])
        s_step(0, 5)
        vs_copy(0)
        qk_group(1, 0)
        qk_group(1, 1)
        po[0] = ps_o.tile([H + 1, T], F32, name="po0", tag="ps_o")
        s_step(1, 0)
        av_j(0, 0)
        s_step(1, 1)
        av_j(0, 1)
        v_group(1, [0, 1])
        s_step(1, 2)
        av_j(0, 2)
        v_group(1, [2, 3])
        s_step(1, 3)
        av_j(0, 3)
        v_group(1, [4, 5])
        s_step(1, 4)
        av_j(0, 4)
        av_j(0, 5)
        v_group(1, [6, 7])
        s_step(1, 5)
        av_j(0, 6)
        av_j(0, 7)
        vs_copy(1)
        finish(0)
        po[1] = ps_o.tile([H + 1, T], F32, name="po1", tag="ps_o")
        for j in range(TT):
            av_j(1, j)
        finish(1)

    nc.finalize()
    return nc


_NC_CACHE = {}


def _get_nc():
    if "nc" not in _NC_CACHE:
        _NC_CACHE["nc"] = build_nc()
    return _NC_CACHE["nc"]


def _make_in_maps(inputs):
    x = np.asarray(inputs["x"], dtype=np.float32)
    wq = np.asarray(inputs["Wq"], dtype=np.float32)
    wk = np.asarray(inputs["Wk"], dtype=np.float32)
    wv = np.asarray(inputs["Wv"], dtype=np.float32)
    # host-side input marshaling: transpose + cast into device layouts
    xT = np.ascontiguousarray(x.transpose(0, 2, 1)).astype(NP_BF16)    # [B, D, T]
    w = np.ascontiguousarray(
        np.concatenate([wq.T, wk.T, wv.T], axis=1)
    ).astype(NP_BF16)                                                  # [D, 3H]
    in_maps = []
    for c in range(NCORES):
        in_maps.append(
            {
                "xT": np.ascontiguousarray(xT[c * BL:(c + 1) * BL]),
                "w": w,
            }
        )
    return in_maps


def _assemble(results):
    # gather: device returns out^T with the softmax denominator as row H;
    # normalize and transpose back to [B, T, H]
    oT = np.concatenate([np.asarray(r["outT"], np.float32) for r in results], axis=0)
    o = oT[:, :H, :] / oT[:, H:H + 1, :]
    return np.ascontiguousarray(o.transpose(0, 2, 1)).astype(np.float32)


def kernel(**inputs):
    from concourse.bass_utils import run_bass_kernel_spmd

    nc = _get_nc()
    res = run_bass_kernel_spmd(nc, _make_in_maps(inputs), list(range(NCORES)))
    return _assemble(res.results)


if __name__ == "__main__":
    nc = build_nc()
    print("built OK")
